# revision 16
# baseline (speedup 1.0000x reference)
"""Trainium2 Bass kernel for nn_CapsuleLayer (conv capsule layer with dynamic routing).

Full (unsharded) inputs in, full output out. Sharding: data-parallel over the
num_capsules axis A=32 -> 8 cores x 4 capsules each (x windows replicated).

v4.3: priors are NEVER materialized (saves v3's entire phase A: 152us of
FD=256 matmuls + 77us LDWEIGHTS + 120us strided ACT copies + 166KB/part
of SBUF). Per 128-row unit ((a,p) rows):

  s0 (PE, runs 2 units ahead): sT0[d, r] = sum_n priors / N via 18
      accumulating matmuls per segment (wde stationary, LDW = 16 cols),
      one transpose back to p-major, SBUF hop so PSUM frees immediately.
  per routing iter:
    g-side on PE (VW trick): vT = transpose(v); VW[r, (ch,c,g)] =
        vT^T @ Wg in two 1152-col PSUM halves (ACT-drained to f16);
        then g = sum_c xws * VW: one 2x DVE mul + 3-add tree over c.
        This replaces v3's 16 per-d muls + add tree (the dominant DVE
        + ACT cost, ~10us/unit-iter across engines).
    softmax: DVE max-reduce (negate) -> ACT exp(bias=-max, accum=sum).
    s-side as v3 mode-d: EX = e*xw (one 2x DVE mul over the slab), 18
        PE transposes through one bank-aligned f16 PSUM tile, 2 DVE
        half-drains, 18 shared-weight matmuls accumulate into psv.
    squash: sqrt(x) = exp(0.5*ln(x)) on the pre-placed natural_log_exp
        table (no table reloads).

History: v3 524us (priors+16-mul g-side); v4 500us (VW g-side + repl
s-side, PE-bound, HAM stuck cold); v4.1 554us (d-major s-matmuls lose
at cold clock); v4.2 failed BIR verification (broadcast-AP LDWEIGHTS
is illegal: "RHS AP can only have one free dimension").
"""
import os
import numpy as np

import concourse.bass as bass
import concourse.bacc as bacc
import concourse.mybir as mybir
import concourse.tile as tile
from concourse.bass_utils import run_bass_kernel_spmd

# problem constants (hardcoded per contract)
K = 3
B, Ci, H, Wd, Cin = 4, 32, 14, 14, 8
A, N, D = 32, 288, 16
w = 12
P = B * w * w           # 576 positions
G = 16                  # route nodes per chunk
CH = N // G             # 18 chunks; G*Cin = 128 = contraction per chunk
AA = A // 8             # capsules per core
NU = (AA * P) // 128    # 18 units of 128 (a,p) rows
HC = CH // 2            # 9 chunks per drain half
HW_ = HC * G * Cin      # 1152 columns per half

F32 = mybir.dt.float32
F16 = mybir.dt.float16
AL = mybir.AluOpType
AF = mybir.ActivationFunctionType
AX = mybir.AxisListType

LAST_RESULT = None

_prog_cache = {}


def _slab_slot(u):
    um = u % 9
    if um <= 3:
        return um
    if um == 4:
        return 8
    return 4 + (um - 5)


def _segments(u):
    """Unit u covers flattened (a,p) rows [128u, 128u+128).
    Returns [(a, p0, rowofs, cnt)]; ro is always 0 or 64."""
    segs = []
    r = u * 128
    end = r + 128
    while r < end:
        a = r // P
        p0 = r % P
        cnt = min(end - r, P - p0)
        segs.append((a, p0, r - u * 128, cnt))
        r += cnt
    return segs


def _build_program():
    key = ("v43",)
    if key in _prog_cache:
        return _prog_cache[key]

    nc = bacc.Bacc()
    xwt_d = nc.dram_tensor("xwt", [128, CH, P], F16, kind="ExternalInput")
    # xw in p-rows at 9 alignments, free order (ch, c, g)
    xws_d = nc.dram_tensor("xws", [128, 9, CH, Cin, G], F16,
                           kind="ExternalInput")
    # VW moving operand, cols (ch, c, g), contraction d=16
    wgd_d = nc.dram_tensor("wgd", [16, AA, CH * Cin * G], F16,
                           kind="ExternalInput")
    # (g,c)-partition weights: s0 stationary
    wde_d = nc.dram_tensor("wde", [128, AA, CH, D], F16, kind="ExternalInput")
    # (c,g)-partition weights: s-matmul moving
    wst_d = nc.dram_tensor("wst", [128, AA, CH, D], F16, kind="ExternalInput")
    bunit_d = nc.dram_tensor("bunit", [128, NU, D], F32, kind="ExternalInput")
    out_d = nc.dram_tensor("out", [AA, P, D], F32, kind="ExternalOutput")

    with tile.TileContext(nc) as tc:
        with (
            tc.tile_pool(name="const", bufs=1) as cp,
            tc.tile_pool(name="sbig", bufs=2) as tp,
            tc.tile_pool(name="lg", bufs=2) as lp,
            tc.tile_pool(name="sm", bufs=3) as sp,
            tc.tile_pool(name="psum_vw", bufs=1, space="PSUM") as qv,
            tc.tile_pool(name="psum_eb", bufs=1, space="PSUM") as qe,
            tc.tile_pool(name="psum_sv", bufs=1, space="PSUM") as qs,
            tc.tile_pool(name="psum_xp", bufs=1, space="PSUM") as qx,
        ):
            nc.scalar.add_instruction(mybir.InstLoadActFuncSet(
                name=nc.get_next_instruction_name(),
                act_func_set_id=6,  # natural_log_exp_and_others
                ins=[], outs=[]))

            wde = cp.tile([128, AA, CH, D], F16)
            nc.sync.dma_start(wde[:], wde_d[:])
            xwt = cp.tile([128, CH, P], F16)
            nc.sync.dma_start(xwt[:], xwt_d[:])
            wst = cp.tile([128, AA, CH, D], F16)
            nc.sync.dma_start(wst[:], wst_d[:])
            bunit = cp.tile([128, NU, D], F32)
            nc.sync.dma_start(bunit[:], bunit_d[:])
            wgs = cp.tile([16, AA, CH * Cin * G], F16)
            nc.sync.dma_start(wgs[:], wgd_d[:])
            from concourse.masks import make_identity
            ident = cp.tile([128, 128], F16)
            make_identity(nc, ident[:])
            xws = cp.tile([128, 9, CH, Cin, G], F16)
            for sl in (0, 1, 2, 3, 8, 4, 5, 6, 7):
                nc.sync.dma_start(xws[:, sl], xws_d[:, sl])

            def squash_pre(s, sq):
                junk = sp.tile([128, D], F32, tag="sqjunk")
                sn = sp.tile([128, 1], F32, tag="sn" + sq)
                nc.vector.scalar_tensor_tensor(
                    out=junk[:], in0=s[:], scalar=1.0, in1=s[:],
                    op0=AL.mult, op1=AL.mult, accum_out=sn[:])
                u1 = sp.tile([128, 1], F32, tag="u1")
                nc.vector.tensor_scalar_add(u1[:], sn[:], 1.0)
                r = sp.tile([128, 1], F32, tag="r" + sq)
                nc.vector.reciprocal(r[:], u1[:])
                return sn, r

            def squash_act(sn, sq):
                t = sp.tile([128, 1], F32, tag="t")
                nc.scalar.activation(t[:], sn[:], AF.Ln)
                rt = sp.tile([128, 1], F32, tag="rt" + sq)
                nc.scalar.activation(rt[:], t[:], AF.Exp, scale=0.5)
                return rt

            def squash_post(s, rt, r, sq, dt):
                f = sp.tile([128, 1], F32, tag="f")
                nc.gpsimd.tensor_mul(f[:], rt[:], r[:])
                o = sp.tile([128, D], dt, tag="o" + sq)
                nc.vector.tensor_scalar_mul(o[:], s[:], f[:])
                return o

            s0_done = {}

            def s0_gen(u):
                """s0 back-transposed to p-major SBUF, emitted 2 units ahead
                so its PE matmuls fill gaps in the routing units' PE stream."""
                segs = _segments(u)
                s0T = qs.tile([16, 128], F32, tag="psv")
                for (a, p0, ro, cnt) in segs:
                    for ch in range(CH):
                        nc.tensor.matmul(
                            s0T[:, ro:ro + cnt],
                            wde[:, a, ch, :],
                            xwt[:, ch, p0:p0 + cnt],
                            start=(ch == 0), stop=(ch == CH - 1))
                yield
                s0s = sp.tile([16, 128], F16, tag="s0s")
                nc.scalar.activation(s0s[:], s0T[:], AF.Copy, scale=1.0 / N)
                yield
                s0p = qx.tile([128, 16], F16, tag="xp")
                nc.tensor.transpose(s0p[:], s0s[:], ident[0:16, 0:16])
                yield
                s0b = sp.tile([128, D], F16, tag="s0b")
                nc.scalar.activation(s0b[:], s0p[:], AF.Copy)
                s0_done[u] = s0b
                yield

            def routing_gen(u):
                segs = _segments(u)
                slab = _slab_slot(u)
                bu = bunit[:, u, :]

                s0b = s0_done.pop(u)
                s = sp.tile([128, D], F32, tag="s0")
                nc.vector.scalar_tensor_tensor(
                    out=s[:], in0=s0b[:], scalar=1.0, in1=bu,
                    op0=AL.mult, op1=AL.add)
                sn, r = squash_pre(s, "0")
                yield
                rt = squash_act(sn, "0")
                yield
                ov = squash_post(s, rt, r, "0", F16)

                lg_prev = None
                for it in (1, 2):
                    sq = str(it)
                    # ---- g-side: vT -> VW halves on PE -> mul+tree on DVE
                    vTp = qx.tile([16, 128], F16, tag="xp")
                    nc.tensor.transpose(vTp[:], ov[:], ident[:])
                    yield
                    vTs = sp.tile([16, 128], F16, tag="vts")
                    nc.scalar.activation(vTs[:], vTp[:], AF.Copy)
                    yield
                    vws = tp.tile([128, CH, Cin, G], F16, tag="vws")
                    vwsf = vws[:].rearrange("p c i g -> p (c i g)")
                    for h in range(2):
                        vw = qv.tile([128, HW_], F32, tag="vwh")
                        for (a, p0, ro, cnt) in segs:
                            for c0 in (0, 512, 1024):
                                cw = min(512, HW_ - c0)
                                nc.tensor.matmul(
                                    vw[ro:ro + cnt, c0:c0 + cw],
                                    vTs[:, ro:ro + cnt],
                                    wgs[:, a, h * HW_ + c0:h * HW_ + c0 + cw],
                                    start=True, stop=True)
                        yield
                        nc.scalar.activation(
                            vwsf[:, h * HW_:(h + 1) * HW_], vw[:], AF.Copy)
                    yield
                    # g = sum_c xws * VW: one 2x mul + halving tree over c
                    gv = tp.tile([128, CH, Cin, G], F16, tag="gv")
                    nc.vector.tensor_mul(gv[:], xws[:, slab], vws[:])
                    gt = tp.tile([128, CH, Cin // 2, G], F16, tag="gt")
                    nc.vector.tensor_add(gt[:], gv[:, :, 0:4], gv[:, :, 4:8])
                    nc.vector.tensor_add(gv[:, :, 0:2], gt[:, :, 0:2],
                                         gt[:, :, 2:4])
                    lg = lp.tile([128, N], F16, tag="lg" + sq)
                    lgv = lg[:].rearrange("p (c g) -> p c g", g=G)
                    if lg_prev is None:
                        nc.vector.tensor_add(lgv, gv[:, :, 0], gv[:, :, 1])
                    else:
                        nc.vector.tensor_add(gt[:, :, 0], gv[:, :, 0],
                                             gv[:, :, 1])
                        nc.vector.tensor_add(
                            lgv, gt[:, :, 0],
                            lg_prev[:].rearrange("p (c g) -> p c g", g=G))
                    lg_prev = lg
                    nmx = sp.tile([128, 1], F32, tag="nmx")
                    nc.vector.tensor_reduce(out=nmx[:], in_=lg[:], axis=AX.X,
                                            op=AL.max, negate=True)
                    yield
                    e = sp.tile([128, N], F16, tag="e")
                    se = sp.tile([128, 1], F32, tag="se")
                    nc.scalar.activation(e[:], lg[:], AF.Exp, bias=nmx[:],
                                         scale=1.0, accum_out=se[:])
                    yield
                    rc = sp.tile([128, 1], F32, tag="rc")
                    nc.vector.reciprocal(rc[:], se[:])
                    # ---- s-side (v3 mode-d): EX = e*xw, 18 PE transposes,
                    # 2 DVE half-drains, 18 shared-weight matmuls
                    ex = tp.tile([128, CH, Cin, G], F16, tag="ex")
                    ev = e[:].rearrange("p (c g) -> p c g", g=G)
                    nc.vector.tensor_mul(
                        ex[:], xws[:, slab],
                        ev[:, :, None, :].broadcast_to([128, CH, Cin, G]))
                    exq = qe.tile([128, CH, 128], F16, tag="exq")
                    for ch in range(CH):
                        nc.tensor.transpose(
                            exq[:, ch, :],
                            ex[:, ch].rearrange("p c g -> p (c g)"),
                            ident[:])
                    yield
                    exts = tp.tile([128, CH, 128], F16, tag="exts")
                    nc.vector.tensor_copy(exts[:, 0:HC], exq[:, 0:HC])
                    nc.vector.tensor_copy(exts[:, HC:CH], exq[:, HC:CH])
                    yield
                    psv = qs.tile([128, 32, D], F32, tag="psv")
                    for (a, p0, ro, cnt) in segs:
                        for ch in range(CH):
                            nc.tensor.matmul(
                                psv[ro:ro + cnt, it, :],
                                exts[:, ch, ro:ro + cnt],
                                wst[:, a, ch, :],
                                start=(ch == 0), stop=(ch == CH - 1))
                    yield
                    s = sp.tile([128, D], F32, tag="s" + sq)
                    nc.vector.scalar_tensor_tensor(
                        out=s[:], in0=psv[:, it, :], scalar=rc[:], in1=bu,
                        op0=AL.mult, op1=AL.add)
                    sn, r = squash_pre(s, sq)
                    yield
                    rt = squash_act(sn, sq)
                    yield
                    ov = squash_post(s, rt, r, sq, F16 if it == 1 else F32)

                for (a, p0, ro, cnt) in segs:
                    nc.sync.dma_start(out_d[a, p0:p0 + cnt, :],
                                      ov[ro:ro + cnt, :])
                yield

            def drain(gens):
                alive = list(gens)
                while alive:
                    nxt = []
                    for g in alive:
                        try:
                            next(g)
                            nxt.append(g)
                        except StopIteration:
                            pass
                    alive = nxt

            drain([s0_gen(0), s0_gen(1)])
            for j in range(0, NU, 2):
                g0 = routing_gen(j)
                g1 = routing_gen(j + 1)
                next(g0)
                next(g0)
                gens = [g0, g1]
                if j + 2 < NU:
                    gens.append(s0_gen(j + 2))
                if j + 3 < NU:
                    gens.append(s0_gen(j + 3))
                drain(gens)

    nc.finalize()
    _prog_cache[key] = nc
    return nc


def _host_prep(x, route_weights, bias):
    x = np.ascontiguousarray(x, dtype=np.float32)
    Wfull = np.ascontiguousarray(route_weights, dtype=np.float32)
    bias = np.ascontiguousarray(bias, dtype=np.float32)

    xw = np.empty((B, w, w, Ci, K, K, Cin), np.float32)
    for ki in range(K):
        for kj in range(K):
            xw[:, :, :, :, ki, kj, :] = (
                x[:, :, ki:ki + w, kj:kj + w, :].transpose(0, 2, 3, 1, 4))
    xw = xw.reshape(P, N, Cin)

    xw4 = xw.reshape(P, CH, G, Cin)
    # xwt[(g,c), ch, p]
    xwt_h = np.ascontiguousarray(
        xw4.transpose(2, 3, 1, 0).reshape(128, CH, P)).astype(np.float16)

    # xws[p-row, slab, ch, c, g] at the 9 unit alignments
    xw_cng = np.ascontiguousarray(xw4.transpose(0, 1, 3, 2))  # [P, CH, c, g]
    rows = np.arange(128)
    xws_h = np.zeros((128, 9, CH, Cin, G), np.float16)
    for q in range(4):
        xws_h[:, q] = xw_cng[128 * q + rows]
    for kk in range(4):
        xws_h[:, 4 + kk] = xw_cng[64 + 128 * kk + rows]
    xws_h[:, 8] = xw_cng[(512 + rows) % P]

    Wn = Wfull.reshape(A, CH, G, Cin, D)
    # wgd[d, a, (ch, c, g)]
    wgd_h = np.ascontiguousarray(
        Wn.transpose(4, 0, 1, 3, 2).reshape(D, A, CH * Cin * G)
    ).astype(np.float16)
    # wde[(g,c), a, ch, d]
    wde_h = np.ascontiguousarray(
        Wn.transpose(2, 3, 0, 1, 4).reshape(128, A, CH, D)).astype(np.float16)
    # wst[(c,g), a, ch, d]
    wst_h = np.ascontiguousarray(
        Wn.transpose(3, 2, 0, 1, 4).reshape(128, A, CH, D)).astype(np.float16)

    in_maps = []
    for k in range(8):
        a0 = k * AA
        bunit_h = np.empty((128, NU, D), np.float32)
        for u in range(NU):
            rr = np.arange(u * 128, u * 128 + 128)
            bunit_h[:, u, :] = bias[a0 + rr // P]
        im = {
            "xwt": xwt_h,
            "xws": xws_h,
            "wgd": np.ascontiguousarray(wgd_h[:, a0:a0 + AA]),
            "wde": np.ascontiguousarray(wde_h[:, a0:a0 + AA]),
            "wst": np.ascontiguousarray(wst_h[:, a0:a0 + AA]),
            "bunit": bunit_h,
        }
        in_maps.append(im)
    return in_maps


def kernel(x, route_weights, bias):
    global LAST_RESULT
    nc = _build_program()
    in_maps = _host_prep(x, route_weights, bias)
    trace = bool(os.environ.get("KERNEL_TRACE"))
    res = run_bass_kernel_spmd(nc, in_maps, list(range(8)), trace=trace)
    LAST_RESULT = res
    full = np.stack([res.results[k]["out"] for k in range(8)])  # [8, AA, P, D]
    full = full.reshape(A, B, w, w, D)
    return np.ascontiguousarray(full.transpose(1, 0, 2, 3, 4))
